# revision 9
# baseline (speedup 1.0000x reference)
"""Trainium2 Bass kernel for nn_AtenMmQuint8: quint8 dense matmul.

    out = ((x - 65) * 0.199) @ ((y - 160) * 0.0215)
    x: [2048, 4096] int32 (quint8 values 0..255)
    y: [4096, 2048] int32 (quint8 values 0..255)
    out: [2048, 2048] fp32

Sharding: 4x2 tensor-parallel grid over the 8 NeuronCores (4 M-blocks x
2 N-blocks); each core computes a 512x1024 output block with full K.

Math: fp8 DoubleRow matmul (2x PE throughput over bf16) with an exact
rank-1 correction that rides for free in the output pass:

    (x-65)@(y-160) = x~@(y~ - 32) + eps
                   = x~@y~ - 32*rowsum(x~)[m] + eps

where x~ = fp8e4m3(x-65) (so the x-side zero-point offset is zero and
no per-column correction is needed) and y~ = fp8e4m3(y-128) (centered
so |v|<=128 and the y-side quant error is halved; its -32 offset folds
into the per-output-row correction corrA[m] = -32*rowsum(x~)[m],
computed on the host from the *quantized* x). The remaining eps is
zero-mean fp8 cross-term noise: measured max rel err 1.18e-2 on the
actual problem seed (threshold 2e-2). All products and partial sums
are integers < 2^24, so fp32 PSUM accumulation is bit-exact and the
host-side error simulation is a faithful predictor of hardware.

Host staging: x is staged K-major (transposed) in fp8 bytes; y in fp8
bytes; corrA ([128, 4] fp32, corrA[p, mg] for output row mg*128+p)
rides along as a tiny side input.

Device kernel (identical SPMD program on all 8 cores):
  - K interleaved across SBUF partitions (k = p*32 + j) so each load
    chunk is 128 large contiguous runs (one per partition); HWDGE
    descriptor generation otherwise serializes the load stream.
  - Progressively-sized load chunks (x on the SP ring, y on the ACT
    ring in parallel); per-ring DMA completions serialize at ~2.2us,
    so the first chunk is minimal (one k-pair) to unlock the stream.
    corrA loads last on the SP ring (needed only at output time).
  - PE prewarm (~30 throwaway matmuls, gated only on a VectorE memset
    so it starts almost immediately) releases the HAM clock gate
    (1.2 -> 2.4 GHz) during the initial load window.
  - Main stream: 128 DoubleRow fp8 matmuls (16 k-pair-tiles x 4 m x
    2 n), each contracting 2 k-tiles (2x128 virtual K) per pass at
    ~238 ns -- 2x the bf16 matmul roofline. The pair dim is an AP dim
    (stride % 16 == 0), no byte interleave needed. The last
    k-pair-tiles run m-major (and the last m-group n-major) so banks
    retire early and correction+scale+store overlaps the remaining
    matmuls.
  - PSUM -> SBUF on VectorE as a single fused tensor_scalar:
    out = (psum + corrA[p]) * SCALE, then one store DMA per 128-row
    group; the last bank's pass and store are split so the
    kernel-ending chain (last matmul -> copy -> store) is short.
"""

import numpy as np
import ml_dtypes

import concourse.bass as bass  # noqa: F401  (kept for callers/debugging)
import concourse.mybir as mybir
import concourse.tile as tile
from concourse import bacc
from concourse.bass_utils import run_bass_kernel_spmd

X_ZP, Y_ZP = 65.0, 160.0
SCALE = 0.199 * 0.0215
Y_CENTER = 128.0
YOFF = Y_CENTER - Y_ZP  # -32: y-160 = (y-128) - 32

M, K, N = 2048, 4096, 2048
GM, GN = 4, 2  # core grid: 4 M-blocks x 2 N-blocks
MC, NC = M // GM, N // GN  # 512 x 1024 per-core output block
P = 128  # partitions / k-tile size
NB = 512  # psum bank free size (one fp32 bank; matmul cannot cross banks)
# k-pair-tiles (2 k-tiles each) per load DMA. Per-ring DMA completions
# serialize at ~2.2us, so small leading chunks start the pipeline and
# big trailing chunks amortize.
DMA_CHUNKS = (1, 2, 3, 3, 4, 3)
JT_TAIL = 8  # trailing k-pair-tiles run m-major so PSUM banks retire early
# Prewarm fills the PE-idle window until the first chunk's DMA
# completion lands (~5us): covers the HAM ramp exactly.
N_WARM = 30


def _emit(tc, xq, yq, corrA, out, dma_chunks=DMA_CHUNKS,
          jt_tail=JT_TAIL, n_warm=N_WARM):
    """Emit the per-core device program.

    xq: [k, mc] f8 DRAM (x~ slice, K-major), yq: [k, nnc] f8 DRAM,
    corrA: [P, mt] fp32 DRAM, out: [mc, nnc] fp32 DRAM.
    """
    nc = tc.nc
    k, mc = xq.shape
    nnc = yq.shape[1]
    kt = k // P
    jt = kt // 2  # DoubleRow k-pair tiles
    mt = mc // P
    nt = nnc // NB
    assert 2 * sum(dma_chunks) == kt

    fp32 = mybir.dt.float32
    bf16 = mybir.dt.bfloat16
    f8 = mybir.dt.float8e4

    with (
        tc.tile_pool(name="sb", bufs=1) as sbp,
        tc.tile_pool(name="osb", bufs=mt, space="SBUF") as osbp,
        tc.tile_pool(name="ps", bufs=mt * nt, space="PSUM") as psp,
    ):
        # Everything is persistent (fits in SBUF at this problem size).
        xs = sbp.tile([P, kt, mc], f8, name="xs")
        ys = sbp.tile([P, kt, nnc], f8, name="ys")
        ca = sbp.tile([P, mt], fp32, name="ca")
        cas = sbp.tile([P, mt], fp32, name="cas")
        wt = sbp.tile([P, P], bf16, name="wt")
        psum = [
            [psp.tile([P, NB], fp32, tag="ps", name=f"ps_{m}_{n}") for n in range(nt)]
            for m in range(mt)
        ]

        # HAM prewarm: the PE sits idle while the first chunks load;
        # throwaway matmuls release the clock gate to 8/8 before the
        # real stream starts. Gate only on a cheap VectorE memset.
        nc.vector.memset(wt[:], 0.0)
        for _ in range(n_warm):
            nc.tensor.matmul(psum[0][0][:, :P], wt[:], wt[:], start=True, stop=True)

        # K is interleaved across partitions (k = p*kt + j): each
        # partition's j-range is one big contiguous DRAM run, so a chunk
        # DMA is 128 descriptors (one per partition). The contraction is
        # a permutation of K, identical for x and y, so the matmul sum
        # is unchanged.
        xqr = xq.rearrange("(p j) m -> p j m", j=kt)
        yqr = yq.rearrange("(p j) n -> p j n", j=kt)
        k0 = 0
        for nj in dma_chunks:
            nk = 2 * nj
            nc.sync.dma_start(xs[:, k0 : k0 + nk, :], xqr[:, k0 : k0 + nk, :])
            # y-loads issue from the ACT HWDGE ring, in parallel with the
            # x-load issues on the SP ring.
            nc.scalar.dma_start(ys[:, k0 : k0 + nk, :], yqr[:, k0 : k0 + nk, :])
            k0 += nk
        # corrA is tiny and only needed by the output pass; load it on
        # the ACT ring after the y chunks so it never delays the stream.
        nc.scalar.dma_start(ca[:], corrA)
        # Pre-scaled copy for the ACT-engine output pass (its activation
        # op computes in*scale + bias, so the bias must carry SCALE).
        nc.vector.tensor_scalar_mul(cas[:], ca[:], SCALE)

        def mm(j2, m, n, nsl=slice(0, NB), psl=slice(0, NB)):
            nc.tensor.matmul(
                psum[m][n][:, psl],
                xs[:, 2 * j2 : 2 * j2 + 2, m * P : (m + 1) * P],
                ys[:, 2 * j2 : 2 * j2 + 2, n * NB + nsl.start : n * NB + nsl.stop],
                start=(j2 == 0),
                stop=(j2 == jt - 1),
                perf_mode=mybir.MatmulPerfMode.DoubleRow,
            )

        # k-outer: touch every psum bank each k-pair-tile so the PE
        # stream stays dense while loads race ahead.
        for j2 in range(jt - jt_tail):
            for m in range(mt):
                for n in range(nt):
                    mm(j2, m, n)
        # m-outer tail: bank group m finishes its K accumulation early so
        # its correction+scale+store overlaps the remaining matmuls. The
        # last m-group runs n-major, and the very last matmul is split
        # in half so the kernel-ending chain is short.
        half = NB // 2
        for m in range(mt):
            if m < mt - 1:
                for j2 in range(jt - jt_tail, jt):
                    for n in range(nt):
                        mm(j2, m, n)
            else:
                for n in range(nt):
                    for j2 in range(jt - jt_tail, jt):
                        if n == nt - 1 and j2 == jt - 1:
                            mm(j2, m, n, nsl=slice(0, half), psl=slice(0, half))
                            mm(j2, m, n, nsl=slice(half, NB), psl=slice(half, NB))
                        else:
                            mm(j2, m, n)

        # Fused correction + scale, PSUM->SBUF: out = (psum + corrA[p])
        # * SCALE on VectorE (ACT helps on the last piece). One store
        # per 128-row group, alternating HWDGE rings so completions
        # overlap; the last group's passes and stores are split so the
        # ending chain (last matmul -> copy -> store) is short.
        def opass(dst, src, m, engine=None):
            (engine or nc.vector).tensor_scalar(
                dst, src, ca[:, m : m + 1], SCALE,
                op0=mybir.AluOpType.add, op1=mybir.AluOpType.mult,
            )

        store_rings = [nc.sync, nc.scalar]
        for m in range(mt):
            osb = osbp.tile([P, nnc], fp32, tag="osb", name=f"osb_{m}")
            ring = store_rings[m % 2]
            if m < mt - 1:
                for n in range(nt):
                    opass(osb[:, n * NB : (n + 1) * NB], psum[m][n][:], m)
                ring.dma_start(out[m * P : (m + 1) * P, :], osb[:])
            else:
                opass(osb[:, 0:NB], psum[m][0][:], m)
                ring.dma_start(out[m * P : (m + 1) * P, 0:NB], osb[:, 0:NB])
                # last bank: first half on DVE (it has higher semaphore
                # pickup latency -- fine for the earlier piece), final
                # half on ACT which starts ~0.15us after its matmul;
                # stores ride different rings in parallel.
                opass(osb[:, NB : NB + half], psum[m][1][:, 0:half], m)
                nc.scalar.dma_start(
                    out[m * P : (m + 1) * P, NB : NB + half],
                    osb[:, NB : NB + half],
                )
                nc.scalar.activation(
                    osb[:, NB + half :],
                    psum[m][1][:, half:],
                    mybir.ActivationFunctionType.Identity,
                    bias=cas[:, m : m + 1],
                    scale=SCALE,
                )
                nc.sync.dma_start(
                    out[m * P : (m + 1) * P, NB + half :],
                    osb[:, NB + half :],
                )


def _build_nc(k=K, mc=MC, nnc=NC, **emit_kw):
    nc = bacc.Bacc("TRN2", target_bir_lowering=False, debug=False)
    xq = nc.declare_dram_parameter("xq", [k, mc], mybir.dt.float8e4, isOutput=False)
    yq = nc.declare_dram_parameter("yq", [k, nnc], mybir.dt.float8e4, isOutput=False)
    corrA = nc.declare_dram_parameter(
        "corrA", [P, mc // P], mybir.dt.float32, isOutput=False
    )
    out = nc.declare_dram_parameter("out", [mc, nnc], mybir.dt.float32, isOutput=True)
    with tile.TileContext(nc) as tc:
        _emit(tc, xq[:], yq[:], corrA[:], out[:], **emit_kw)
    nc.compile()
    return nc


_CACHE = {}


def _get_nc():
    if "nc" not in _CACHE:
        _CACHE["nc"] = _build_nc()
    return _CACHE["nc"]


def kernel(x, y):
    x = np.asarray(x)
    y = np.asarray(y)
    assert x.shape == (M, K) and y.shape == (K, N)
    f8 = ml_dtypes.float8_e4m3
    # fp8 quantization (values guaranteed 0..255 by the spec): x keeps
    # its true zero-point (no per-column correction needed), y centered.
    xq = (x.astype(np.float32) - X_ZP).astype(f8)  # [M, K]
    yq = (y.astype(np.float32) - Y_CENTER).astype(f8)  # [K, N]
    # Exact rank-1 correction computed from the *quantized* x.
    Rx = xq.astype(np.float32).sum(axis=1, dtype=np.float64)  # [M]
    corrA_full = (YOFF * Rx).astype(np.float32)  # [M]
    xqT = np.ascontiguousarray(xq.T)  # [K, M] K-major

    in_maps = []
    for i in range(GM * GN):
        mi, ni = divmod(i, GN)
        in_maps.append(
            {
                "xq": np.ascontiguousarray(xqT[:, mi * MC : (mi + 1) * MC]),
                "yq": np.ascontiguousarray(yq[:, ni * NC : (ni + 1) * NC]),
                # corrA[p, mg] covers output row mg*128 + p of this block
                "corrA": np.ascontiguousarray(
                    corrA_full[mi * MC : (mi + 1) * MC].reshape(MC // P, P).T
                ),
            }
        )

    res = run_bass_kernel_spmd(_get_nc(), in_maps, list(range(GM * GN)))
    _CACHE["last_results"] = res

    out = np.empty((M, N), np.float32)
    for i in range(GM * GN):
        mi, ni = divmod(i, GN)
        out[mi * MC : (mi + 1) * MC, ni * NC : (ni + 1) * NC] = res.results[i]["out"]
    return out


# revision 10
# speedup vs baseline: 1.0311x; 1.0311x over previous
"""Trainium2 Bass kernel for nn_AtenMmQuint8: quint8 dense matmul.

    out = ((x - 65) * 0.199) @ ((y - 160) * 0.0215)
    x: [2048, 4096] int32 (quint8 values 0..255)
    y: [4096, 2048] int32 (quint8 values 0..255)
    out: [2048, 2048] fp32

Sharding: 4x2 tensor-parallel grid over the 8 NeuronCores (4 M-blocks x
2 N-blocks); each core computes a 512x1024 output block with full K.

Math: fp8 DoubleRow matmul (2x PE throughput over bf16) with an exact
rank-1 correction that rides for free in the output pass:

    (x-65)@(y-160) = x~@(y~ - 32) + eps
                   = x~@y~ - 32*rowsum(x~)[m] + eps

where x~ = fp8e4m3(x-65) (so the x-side zero-point offset is zero and
no per-column correction is needed) and y~ = fp8e4m3(y-128) (centered
so |v|<=128 and the y-side quant error is halved; its -32 offset folds
into the per-output-row correction corrA[m] = -32*rowsum(x~)[m],
computed on the host from the *quantized* x). The remaining eps is
zero-mean fp8 cross-term noise: measured max rel err 1.18e-2 on the
actual problem seed (threshold 2e-2). All products and partial sums
are integers < 2^24, so fp32 PSUM accumulation is bit-exact and the
host-side error simulation is a faithful predictor of hardware.

Host staging: x is staged K-major (transposed) in fp8 bytes; y in fp8
bytes; corrA ([128, 4] fp32, corrA[p, mg] for output row mg*128+p)
rides along as a tiny side input.

Device kernel (identical SPMD program on all 8 cores):
  - K interleaved across SBUF partitions (k = p*32 + j) so each load
    chunk is 128 large contiguous runs (one per partition); HWDGE
    descriptor generation otherwise serializes the load stream.
  - Progressively-sized load chunks (x on the SP ring, y on the ACT
    ring in parallel); per-ring DMA completions serialize at ~2.2us,
    so the first chunk is minimal (one k-pair) to unlock the stream.
    corrA loads last on the SP ring (needed only at output time).
  - PE prewarm (~30 throwaway matmuls, gated only on a VectorE memset
    so it starts almost immediately) releases the HAM clock gate
    (1.2 -> 2.4 GHz) during the initial load window.
  - Main stream: 128 DoubleRow fp8 matmuls (16 k-pair-tiles x 4 m x
    2 n), each contracting 2 k-tiles (2x128 virtual K) per pass at
    ~238 ns -- 2x the bf16 matmul roofline. The pair dim is an AP dim
    (stride % 16 == 0), no byte interleave needed. The last
    k-pair-tiles run m-major (and the last m-group n-major) so banks
    retire early and correction+scale+store overlaps the remaining
    matmuls.
  - PSUM -> SBUF on VectorE as a single fused tensor_scalar:
    out = (psum + corrA[p]) * SCALE, then one store DMA per 128-row
    group; the last bank's pass and store are split so the
    kernel-ending chain (last matmul -> copy -> store) is short.
"""

import numpy as np
import ml_dtypes

import concourse.bass as bass  # noqa: F401  (kept for callers/debugging)
import concourse.mybir as mybir
import concourse.tile as tile
from concourse import bacc
from concourse.bass_utils import run_bass_kernel_spmd

X_ZP, Y_ZP = 65.0, 160.0
SCALE = 0.199 * 0.0215
Y_CENTER = 128.0
YOFF = Y_CENTER - Y_ZP  # -32: y-160 = (y-128) - 32

M, K, N = 2048, 4096, 2048
GM, GN = 4, 2  # core grid: 4 M-blocks x 2 N-blocks
MC, NC = M // GM, N // GN  # 512 x 1024 per-core output block
P = 128  # partitions / k-tile size
NB = 512  # psum bank free size (one fp32 bank; matmul cannot cross banks)
# k-pair-tiles (2 k-tiles each) per load DMA. Per-ring DMA completions
# serialize at ~2.2us, so small leading chunks start the pipeline and
# big trailing chunks amortize.
DMA_CHUNKS = (1, 2, 3, 3, 4, 3)
JT_TAIL = 8  # trailing k-pair-tiles run m-major so PSUM banks retire early
# Prewarm fills the PE-idle window until the first chunk's DMA
# completion lands (~5us): covers the HAM ramp exactly.
N_WARM = 34


def _emit(tc, xq, yq, corrA, out, dma_chunks=DMA_CHUNKS,
          jt_tail=JT_TAIL, n_warm=N_WARM):
    """Emit the per-core device program.

    xq: [k, mc] f8 DRAM (x~ slice, K-major), yq: [k, nnc] f8 DRAM,
    corrA: [P, mt] fp32 DRAM, out: [mc, nnc] fp32 DRAM.
    """
    nc = tc.nc
    k, mc = xq.shape
    nnc = yq.shape[1]
    kt = k // P
    jt = kt // 2  # DoubleRow k-pair tiles
    mt = mc // P
    nt = nnc // NB
    assert 2 * sum(dma_chunks) == kt

    fp32 = mybir.dt.float32
    bf16 = mybir.dt.bfloat16
    f8 = mybir.dt.float8e4

    with (
        tc.tile_pool(name="sb", bufs=1) as sbp,
        tc.tile_pool(name="osb", bufs=mt, space="SBUF") as osbp,
        tc.tile_pool(name="ps", bufs=mt * nt, space="PSUM") as psp,
    ):
        # Everything is persistent (fits in SBUF at this problem size).
        xs = sbp.tile([P, kt, mc], f8, name="xs")
        ys = sbp.tile([P, kt, nnc], f8, name="ys")
        ca = sbp.tile([P, mt], fp32, name="ca")
        cas = sbp.tile([P, mt], fp32, name="cas")
        wt = sbp.tile([P, P], bf16, name="wt")
        psum = [
            [psp.tile([P, NB], fp32, tag="ps", name=f"ps_{m}_{n}") for n in range(nt)]
            for m in range(mt)
        ]

        # HAM prewarm: the PE sits idle while the first chunks load;
        # throwaway matmuls release the clock gate to 8/8 before the
        # real stream starts. Gate only on a cheap VectorE memset.
        nc.vector.memset(wt[:], 0.0)
        for _ in range(n_warm):
            nc.tensor.matmul(psum[0][0][:, :P], wt[:], wt[:], start=True, stop=True)

        # K is interleaved across partitions (k = p*kt + j): each
        # partition's j-range is one big contiguous DRAM run, so a chunk
        # DMA is 128 descriptors (one per partition). The contraction is
        # a permutation of K, identical for x and y, so the matmul sum
        # is unchanged.
        xqr = xq.rearrange("(p j) m -> p j m", j=kt)
        yqr = yq.rearrange("(p j) n -> p j n", j=kt)
        k0 = 0
        for nj in dma_chunks:
            nk = 2 * nj
            nc.sync.dma_start(xs[:, k0 : k0 + nk, :], xqr[:, k0 : k0 + nk, :])
            # y-loads issue from the ACT HWDGE ring, in parallel with the
            # x-load issues on the SP ring.
            nc.scalar.dma_start(ys[:, k0 : k0 + nk, :], yqr[:, k0 : k0 + nk, :])
            k0 += nk
        # corrA is tiny and only needed by the output pass; load it on
        # the ACT ring after the y chunks so it never delays the stream.
        nc.scalar.dma_start(ca[:], corrA)
        # Pre-scaled copy for the ACT-engine output pass (its activation
        # op computes in*scale + bias, so the bias must carry SCALE).
        nc.vector.tensor_scalar_mul(cas[:], ca[:], SCALE)

        def mm(j2, m, n, nsl=slice(0, NB), psl=slice(0, NB)):
            nc.tensor.matmul(
                psum[m][n][:, psl],
                xs[:, 2 * j2 : 2 * j2 + 2, m * P : (m + 1) * P],
                ys[:, 2 * j2 : 2 * j2 + 2, n * NB + nsl.start : n * NB + nsl.stop],
                start=(j2 == 0),
                stop=(j2 == jt - 1),
                perf_mode=mybir.MatmulPerfMode.DoubleRow,
            )

        # k-outer: touch every psum bank each k-pair-tile so the PE
        # stream stays dense while loads race ahead.
        for j2 in range(jt - jt_tail):
            for m in range(mt):
                for n in range(nt):
                    mm(j2, m, n)
        # m-outer tail: bank group m finishes its K accumulation early so
        # its correction+scale+store overlaps the remaining matmuls. The
        # last m-group runs n-major, and the very last matmul is split
        # in half so the kernel-ending chain is short.
        half = NB // 2
        for m in range(mt):
            if m < mt - 1:
                for j2 in range(jt - jt_tail, jt):
                    for n in range(nt):
                        mm(j2, m, n)
            else:
                for n in range(nt):
                    for j2 in range(jt - jt_tail, jt):
                        if n == nt - 1 and j2 == jt - 1:
                            mm(j2, m, n, nsl=slice(0, half), psl=slice(0, half))
                            mm(j2, m, n, nsl=slice(half, NB), psl=slice(half, NB))
                        else:
                            mm(j2, m, n)

        # Fused correction + scale, PSUM->SBUF: out = (psum + corrA[p])
        # * SCALE on VectorE (ACT helps on the last piece). One store
        # per 128-row group, alternating HWDGE rings so completions
        # overlap; the last group's passes and stores are split so the
        # ending chain (last matmul -> copy -> store) is short.
        def opass(dst, src, m, engine=None):
            (engine or nc.vector).tensor_scalar(
                dst, src, ca[:, m : m + 1], SCALE,
                op0=mybir.AluOpType.add, op1=mybir.AluOpType.mult,
            )

        # Ring/queue assignment keeps the kernel-ending chain unblocked:
        # the ACT engine's strict-FIFO queue must not hold any store
        # whose wait (on a DVE pass) would delay the final activation.
        store_rings = [nc.scalar, nc.sync]
        for m in range(mt):
            osb = osbp.tile([P, nnc], fp32, tag="osb", name=f"osb_{m}")
            ring = store_rings[m % 2]
            last_ring = nc.sync
            if m < mt - 1:
                for n in range(nt):
                    opass(osb[:, n * NB : (n + 1) * NB], psum[m][n][:], m)
                ring.dma_start(out[m * P : (m + 1) * P, :], osb[:])
            else:
                opass(osb[:, 0:NB], psum[m][0][:], m)
                last_ring.dma_start(out[m * P : (m + 1) * P, 0:NB], osb[:, 0:NB])
                # last bank: first half on DVE (it has higher semaphore
                # pickup latency -- fine for the earlier piece, store on
                # the SP ring), final half on ACT which starts ~0.15us
                # after its matmul and feeds its own ring's store.
                opass(osb[:, NB : NB + half], psum[m][1][:, 0:half], m)
                last_ring.dma_start(
                    out[m * P : (m + 1) * P, NB : NB + half],
                    osb[:, NB : NB + half],
                )
                nc.scalar.activation(
                    osb[:, NB + half :],
                    psum[m][1][:, half:],
                    mybir.ActivationFunctionType.Identity,
                    bias=cas[:, m : m + 1],
                    scale=SCALE,
                )
                nc.scalar.dma_start(
                    out[m * P : (m + 1) * P, NB + half :],
                    osb[:, NB + half :],
                )


def _build_nc(k=K, mc=MC, nnc=NC, **emit_kw):
    nc = bacc.Bacc("TRN2", target_bir_lowering=False, debug=False)
    xq = nc.declare_dram_parameter("xq", [k, mc], mybir.dt.float8e4, isOutput=False)
    yq = nc.declare_dram_parameter("yq", [k, nnc], mybir.dt.float8e4, isOutput=False)
    corrA = nc.declare_dram_parameter(
        "corrA", [P, mc // P], mybir.dt.float32, isOutput=False
    )
    out = nc.declare_dram_parameter("out", [mc, nnc], mybir.dt.float32, isOutput=True)
    with tile.TileContext(nc) as tc:
        _emit(tc, xq[:], yq[:], corrA[:], out[:], **emit_kw)
    nc.compile()
    return nc


_CACHE = {}


def _get_nc():
    if "nc" not in _CACHE:
        _CACHE["nc"] = _build_nc()
    return _CACHE["nc"]


def kernel(x, y):
    x = np.asarray(x)
    y = np.asarray(y)
    assert x.shape == (M, K) and y.shape == (K, N)
    f8 = ml_dtypes.float8_e4m3
    # fp8 quantization (values guaranteed 0..255 by the spec): x keeps
    # its true zero-point (no per-column correction needed), y centered.
    xq = (x.astype(np.float32) - X_ZP).astype(f8)  # [M, K]
    yq = (y.astype(np.float32) - Y_CENTER).astype(f8)  # [K, N]
    # Exact rank-1 correction computed from the *quantized* x.
    Rx = xq.astype(np.float32).sum(axis=1, dtype=np.float64)  # [M]
    corrA_full = (YOFF * Rx).astype(np.float32)  # [M]
    xqT = np.ascontiguousarray(xq.T)  # [K, M] K-major

    in_maps = []
    for i in range(GM * GN):
        mi, ni = divmod(i, GN)
        in_maps.append(
            {
                "xq": np.ascontiguousarray(xqT[:, mi * MC : (mi + 1) * MC]),
                "yq": np.ascontiguousarray(yq[:, ni * NC : (ni + 1) * NC]),
                # corrA[p, mg] covers output row mg*128 + p of this block
                "corrA": np.ascontiguousarray(
                    corrA_full[mi * MC : (mi + 1) * MC].reshape(MC // P, P).T
                ),
            }
        )

    res = run_bass_kernel_spmd(_get_nc(), in_maps, list(range(GM * GN)))
    _CACHE["last_results"] = res

    out = np.empty((M, N), np.float32)
    for i in range(GM * GN):
        mi, ni = divmod(i, GN)
        out[mi * MC : (mi + 1) * MC, ni * NC : (ni + 1) * NC] = res.results[i]["out"]
    return out
